# revision 30
# baseline (speedup 1.0000x reference)
"""Causal ReLU-attention block (qkv proj + per-head attention) on 8 trn2 cores.

Sharding: pure data-parallel over batch (B=8 -> 1 batch element per core).
Per-core: x_b [T,C] -> qkv -> scoresT = k q^T (row-tiled head pairs) ->
relu/causal-mask -> yT = v.T @ scoresT (col-tiled head pairs) -> DRAM yT [C,T].
Host side: transpose/cast shards in, transpose gather out.
"""

import sys
from contextlib import ExitStack

sys.path.insert(0, "/opt/trn_rl_repo")

import ml_dtypes
import numpy as np

import concourse.bass as bass
import concourse.tile as tile
from concourse import bacc, bass_utils, mybir

P = 128
QW = 512  # t_q chunk width (PSUM bank = 512 fp32)

BF16 = mybir.dt.bfloat16
F32 = mybir.dt.float32
AF = mybir.ActivationFunctionType
ALU = mybir.AluOpType


def build_module(T=1024, C=768, H=12, n_cores=8):
    """Build + compile the per-core Bass module (same program on all cores)."""
    hd = C // H
    assert hd == 64 and H % 2 == 0 and C % P == 0 and T % QW == 0
    CT = C // P            # contraction tiles over C
    TT = T // P            # t tiles
    NQC = T // QW          # q chunks
    VW = C // 2            # v output free-dim chunk width (<=512)
    NHP = H // 2           # head pairs
    scale = 1.0 / float(np.sqrt(hd))

    nc = bacc.Bacc("TRN2", target_bir_lowering=False, debug=False,
                   num_devices=n_cores)

    xT = nc.dram_tensor("xT", [C, T], BF16, kind="ExternalInput").ap()
    wT = nc.dram_tensor("wT", [C, 3 * C], BF16, kind="ExternalInput").ap()
    bqk = nc.dram_tensor("bqk", [P, 2 * CT], F32, kind="ExternalInput").ap()
    bv = nc.dram_tensor("bv", [P, C], F32, kind="ExternalInput").ap()
    yT = nc.dram_tensor("yT", [C, T], F32, kind="ExternalOutput").ap()

    wT3 = wT.rearrange("(ct p) o -> p ct o", p=P)
    xT3 = xT.rearrange("(ct p) t -> p ct t", p=P)

    with tile.TileContext(nc) as tc, ExitStack() as ctx:
        const = ctx.enter_context(tc.tile_pool(name="const", bufs=1))
        psum = ctx.enter_context(tc.tile_pool(name="psum", bufs=3, space="PSUM"))
        ypsum = ctx.enter_context(tc.tile_pool(name="ypsum", bufs=2, space="PSUM"))
        scb = ctx.enter_context(tc.tile_pool(name="scb", bufs=14))
        ysb = ctx.enter_context(tc.tile_pool(name="ysb", bufs=3))

        # inputs staged in fine chunks, alternating the two HWDGE queues
        # (sync / scalar), ordered by first use
        wt_sb = const.tile([P, CT, 3 * C], BF16)
        xt_sb = const.tile([P, CT, T], BF16)
        for ct in range(CT):
            nc.scalar.dma_start(wt_sb[:, ct, 0:P], wT3[:, ct, 0:P])
            nc.sync.dma_start(xt_sb[:, ct, 0:QW], xT3[:, ct, 0:QW])
        bqk_sb = const.tile([P, 2 * CT], F32)
        nc.scalar.dma_start(bqk_sb[:], bqk[:])
        for ct in range(CT):
            nc.scalar.dma_start(wt_sb[:, ct, P:C], wT3[:, ct, P:C])
            if T > QW:
                nc.sync.dma_start(xt_sb[:, ct, QW:T], xT3[:, ct, QW:T])
        for ct in range(CT):
            eng = nc.sync if ct % 2 == 0 else nc.scalar
            eng.dma_start(wt_sb[:, ct, C:2 * C], wT3[:, ct, C:2 * C])
        bv_sb = const.tile([P, 2, VW], F32)
        nc.scalar.dma_start(bv_sb[:], bv.rearrange("p (oc v) -> p oc v", oc=2))
        for ct in range(CT):
            eng = nc.sync if ct % 2 == 0 else nc.scalar
            eng.dma_start(wt_sb[:, ct, 2 * C:3 * C], wT3[:, ct, 2 * C:3 * C])

        # 0/1 upper-triangle mask (keep j >= p) for DVE-side causal masking
        mask_sb = const.tile([P, QW], BF16)
        nc.gpsimd.memset(mask_sb[:], 1.0)
        nc.gpsimd.affine_select(
            mask_sb[:], mask_sb[:], pattern=[[1, QW]],
            compare_op=ALU.is_ge, fill=0.0, base=0, channel_multiplier=-1)

        qkT = const.tile([P, 2 * CT, T], BF16)   # o-tiles: q = 0..CT-1, k = CT..
        vsb = const.tile([P, TT, C], BF16)       # v in natural [t, o] layout

        evict = [0]
        mstep = [0]

        def relu_evict(dst, src):
            # relu(scale * s): PSUM -> SBUF bf16, alternating ACT / DVE
            if evict[0] % 2 == 0:
                nc.scalar.activation(dst, src, AF.Relu, scale=scale)
            else:
                nc.vector.tensor_scalar(dst, src, scale, 0.0, ALU.mult, ALU.max)
            evict[0] += 1

        def emit_v(tt):
            ps = psum.tile([P, 2, QW], F32, tag="blk", name="v_ps")
            for ct in range(CT):
                for oc in range(2):
                    nc.tensor.matmul(
                        ps[:, oc, :VW],
                        xt_sb[:, ct, tt * P:(tt + 1) * P],
                        wt_sb[:, ct, 2 * C + oc * VW:2 * C + (oc + 1) * VW],
                        start=(ct == 0), stop=(ct == CT - 1),
                    )
            nc.vector.tensor_tensor(
                vsb[:, tt].rearrange("p (oc v) -> p oc v", oc=2),
                ps[:, :, :VW], bv_sb[:], ALU.add)

        def emit_qk(ot):
            # qc-outer so the first accumulation chain only needs the first
            # x chunk of each c-tile
            ps = psum.tile([P, 2, QW], F32, tag="blk", name="qk_ps")
            for qc in range(NQC):
                for ct in range(CT):
                    nc.tensor.matmul(
                        ps[:, qc],
                        wt_sb[:, ct, ot * P:(ot + 1) * P],
                        xt_sb[:, ct, qc * QW:(qc + 1) * QW],
                        start=(ct == 0), stop=(ct == CT - 1),
                    )
            # split the bias-add eviction across ACT and DVE so the PSUM
            # slot frees in half the time
            half = (NQC + 1) // 2
            nc.scalar.activation(
                qkT[:, ot, 0:half * QW],
                ps[:, :half].rearrange("p a b -> p (a b)"),
                AF.Identity, bias=bqk_sb[:, ot:ot + 1])
            if NQC > half:
                nc.vector.tensor_scalar(
                    qkT[:, ot, half * QW:T],
                    ps[:, half:NQC].rearrange("p a b -> p (a b)"),
                    bqk_sb[:, ot:ot + 1], None, ALU.add)

        def attention_closures(hp):
            """Parallel (scores, att@v) emission closures per block step for
            one head pair; the interleaver runs att@v a full super-step
            behind its scores so the FIFO PE queue always has ready work."""
            items = []
            for qc in range(NQC):
                kb_hi = min((qc * QW + QW - 1) // P, TT - 1)
                for kb in range(kb_hi + 1):
                    items.append((qc, kb, kb_hi))
            state = {"s": {}, "y": {}}
            sc_fns, av_fns = [], []

            def sc(i, qc, kb, kb_hi):
                delta = max(kb * P - qc * QW, 0)   # first valid t_q col
                sp = psum.tile([P, 2, QW], F32, tag="blk", name="s_ps")
                for h, ppos in ((0, (0, 0)), (1, (64, 0))):
                    nc.tensor.matmul(
                        sp[:, h, delta:QW],
                        qkT[h * 64:(h + 1) * 64, CT + hp,
                            kb * P:(kb + 1) * P],
                        qkT[h * 64:(h + 1) * 64, hp,
                            qc * QW + delta:(qc + 1) * QW],
                        start=True, stop=True, tile_position=ppos,
                    )
                s = scb.tile([P, 2, QW], BF16, tag="s")
                relu_evict(s[:, :, delta:QW], sp[:, :, delta:QW])
                if kb * P >= qc * QW:   # diagonal block: causal mask on the
                    # first P cols only (row p can only mask j' < p < P)
                    if True:
                        nc.gpsimd.affine_select(
                            s[:, :, delta:delta + P],
                            s[:, :, delta:delta + P],
                            pattern=[[0, 2], [1, P]],
                            compare_op=ALU.is_ge, fill=0.0,
                            base=0, channel_multiplier=-1,
                        )
                    else:
                        nc.vector.tensor_tensor(
                            s[:, :, delta:delta + P],
                            s[:, :, delta:delta + P],
                            mask_sb[:, None, 0:P].to_broadcast((P, 2, P)),
                            ALU.mult,
                        )
                    mstep[0] += 1
                state["s"][i] = s

            def av(i, qc, kb, kb_hi):
                if kb == 0:
                    state["y"][qc] = ypsum.tile([P, QW], F32, tag="y",
                                                name="yp")
                yp = state["y"][qc]
                delta = max(kb * P - qc * QW, 0)
                s = state["s"].pop(i)
                # the two heads accumulate into disjoint partition ranges of
                # one bank (different per-partition SRAMs, so concurrent
                # drains are safe); each runs its own start/stop group (the
                # sim's group checker can't see base partition -> skip)
                nc.tensor.matmul(
                    yp[0:64, delta:QW], vsb[:, kb, hp * P:hp * P + 64],
                    s[:, 0, delta:QW],
                    start=(kb == 0), stop=(kb == kb_hi),
                    tile_position=(0, 0), skip_group_check=True,
                )
                nc.tensor.matmul(
                    yp[64:128, delta:QW],
                    vsb[:, kb, hp * P + 64:hp * P + 128],
                    s[:, 1, delta:QW],
                    start=(kb == 0), stop=(kb == kb_hi),
                    tile_position=(0, 64), skip_group_check=True,
                )
                if kb == kb_hi:
                    yp = state["y"].pop(qc)
                    yt = ysb.tile([P, QW], F32, tag="yt")
                    nc.scalar.activation(yt[0:64, :], yp[0:64, :], AF.Copy)
                    nc.vector.tensor_copy(yt[64:128, :], yp[64:128, :])
                    nc.sync.dma_start(
                        yT[hp * P:(hp + 1) * P, qc * QW:(qc + 1) * QW],
                        yt[:])

            for i, (qc, kb, kb_hi) in enumerate(items):
                sc_fns.append(
                    lambda i=i, qc=qc, kb=kb, kb_hi=kb_hi: sc(i, qc, kb, kb_hi))
                av_fns.append(
                    lambda i=i, qc=qc, kb=kb, kb_hi=kb_hi: av(i, qc, kb, kb_hi))
            return sc_fns, av_fns

        # qk first (gated on small weight slivers, in DMA arrival order),
        # then v, then attention with two head pairs' chains interleaved and
        # att@v lagged one super-step behind its scores
        for ot in range(2 * CT):
            emit_qk(ot)
        for tt in range(TT):
            emit_v(tt)
        groups = [(j, j + 1) for j in range(0, NHP - 1, 2)]
        if NHP % 2:
            groups.append((NHP - 1,))
        for grp in groups:
            streams = [attention_closures(hp) for hp in grp]
            LAG = 3
            n = len(streams[0][0])
            for i in range(n + LAG):
                if i < n:
                    for sc_fns, _ in streams:
                        sc_fns[i]()
                if i >= LAG:
                    for _, av_fns in streams:
                        av_fns[i - LAG]()

    nc.compile()
    return nc


_CACHE = {}


def _get_module():
    if "nc" not in _CACHE:
        _CACHE["nc"] = build_module()
    return _CACHE["nc"]


def _prep_in_maps(x, W_attn, b_attn, T=1024, C=768, n_cores=8):
    bf = ml_dtypes.bfloat16
    OT = 2 * C // P
    WT = np.ascontiguousarray(W_attn.astype(np.float32).T).astype(bf)  # [C, 3C]
    bqk = np.ascontiguousarray(
        b_attn[:2 * C].astype(np.float32).reshape(OT, P).T)            # [P, OT]
    bv = np.ascontiguousarray(
        np.tile(b_attn[2 * C:].astype(np.float32)[None, :], (P, 1)))   # [P, C]
    in_maps = []
    for c in range(n_cores):
        xT_b = np.ascontiguousarray(x[c].astype(np.float32).T).astype(bf)
        in_maps.append({"xT": xT_b, "wT": WT, "bqk": bqk, "bv": bv})
    return in_maps


def run(x, W_attn, b_attn, trace=False):
    nc = _get_module()
    in_maps = _prep_in_maps(x, W_attn, b_attn)
    res = bass_utils.run_bass_kernel_spmd(
        nc, in_maps, core_ids=list(range(8)), trace=trace)
    y = np.stack([np.asarray(res.results[c]["yT"]).T for c in range(8)])
    return np.ascontiguousarray(y.astype(np.float32)), res


def kernel(x, W_attn, b_attn):
    y, _ = run(x, W_attn, b_attn, trace=False)
    return y
